# revision 17
# baseline (speedup 1.0000x reference)
"""MoE (top-2 of 8 experts, D=H=1024) on 8 Trainium2 NeuronCores.

Strategy (expert-parallel, matching the sharding hint):
  - Host computes the router (softmax + top-k + expert-sort dispatch) in
    float64 -- the dispatch/sharding decision, 0.2% of total FLOPs.
  - Tokens are gathered per expert (capacity-padded); core c gets expert c's
    token block plus expert c's weights.
  - Each core runs the 2-layer expert MLP in "transposed activation" layout
    (activations are [feature, token]) so no on-device transposes are needed:
        hT = w_in.T @ xT   (lhsT = w_in chunk, natural layout)
        yT = w_out.T @ hT  (lhsT = w_out chunk, natural layout)
  - Weights/activations stream over BOTH HWDGE queues (SP + Activation) in
    consumption order so the PE is never DMA-starved; activations are
    host-packed [128, KD*C] so every DMA line is contiguous.
  - Layer-2's last two contraction chunks run m-outer so PSUM banks complete
    staggered; each pair is drained (DVE/Pool copy to fp16 + DMA out on
    alternating queues) while the remaining matmuls run.
  - Host scales rows by the gate probability (zero for padding rows) and
    scatter-adds back into the [T, D] output.
"""

import os
import sys

import numpy as np

for _p in ("/opt/trn_rl_repo", "/root/.axon_site/_ro/trn_rl_repo"):
    if os.path.isdir(_p) and _p not in sys.path:
        sys.path.append(_p)


def _ensure_ntff_hook():
    """Register the axon NTFF profiling hook if the image's antenv lacks it."""
    try:
        import antenv.axon_hooks  # noqa: F401

        return
    except ImportError:
        pass
    import types

    try:
        import antenv
    except ImportError:
        return
    mod = types.ModuleType("antenv.axon_hooks")
    _hook = [None]
    mod.set_axon_ntff_profile_hook = lambda h: _hook.__setitem__(0, h)
    mod.get_axon_ntff_profile_hook = lambda: _hook[0]
    sys.modules["antenv.axon_hooks"] = mod
    antenv.axon_hooks = mod
    try:
        from trn_agent_boot.trn_boot import _ntff_profile_via_ctypes

        mod.set_axon_ntff_profile_hook(
            _ntff_profile_via_ctypes("/opt/axon/libaxon_pjrt.so")
        )
    except Exception:
        pass


_ensure_ntff_hook()

D, H, E, TOPK = 1024, 1024, 8, 2
N_CORES = 8
P = 128  # partitions

MM_DTYPE = os.environ.get("MOE_MM_DTYPE", "float16")
NWARM = int(os.environ.get("MOE_NWARM", "28"))

_compiled_cache = {}


def _np_mm_dtype(mm_dtype_str):
    if mm_dtype_str in ("float32", "float32r"):
        return np.float32
    if mm_dtype_str == "float16":
        return np.float16
    if mm_dtype_str == "bfloat16":
        import ml_dtypes

        return np.dtype(ml_dtypes.bfloat16)
    raise ValueError(mm_dtype_str)


def _build_program(C, mm_dtype_str):
    """One expert's MLP over a [C] token block; same program on all cores."""
    from concourse import bacc, mybir, tile

    f32 = mybir.dt.float32
    f16 = mybir.dt.float16
    mm_dt = getattr(mybir.dt, mm_dtype_str)
    nc = bacc.Bacc(None, target_bir_lowering=False, debug=False)

    xT_d = nc.dram_tensor("xT", [P, (D // P) * C], mm_dt, kind="ExternalInput")
    w_in_d = nc.dram_tensor("w_in", [D, H], mm_dt, kind="ExternalInput")
    w_out_d = nc.dram_tensor("w_out", [H, D], mm_dt, kind="ExternalInput")
    # host-packed [p, b*KH+m] = bias[b, m*128+p]: contiguous 64B/partition
    # (the naive [2, H] gather generates 2048 4-byte DMA descriptors that
    # clog the qAct ring for ~3us)
    bias_d = nc.dram_tensor("bias", [P, 2 * (H // P)], f32, kind="ExternalInput")
    yT_d = nc.dram_tensor("yT", [P, (D // P) * C], f16, kind="ExternalOutput")

    KD = D // P  # contraction chunks, layer 1 (and output chunks, layer 2)
    KH = H // P

    # Semaphore/DGE hygiene at program START instead of the tile epilogue:
    # the drain-with-reset walk costs a fixed ~7.7us on hardware.  At the
    # tail it sits on the critical path after the last output DMA; at the
    # head it runs while nothing else is in flight (and each execution
    # cleans before use, so re-execution stays correct).  The pseudo sync
    # barrier is NRT-expanded outside the bass sem range, so it is safe to
    # order engines after a gpsimd-only sem_clear.
    # (measured: moving the walk to the head costs ~2.9us vs the epilogue
    # version -- the walk is fixed-cost and fully serial at the head -- so
    # this stays off)
    start_clean = os.environ.get("MOE_START_CLEAN", "0") == "1"
    if start_clean:
        sem_range = range(*[int(x) for x in os.environ.get(
            "MOE_CLEAN_RANGE", "150,256").split(",")])
        nc.gpsimd.dma_reset(sem_range)
        nc.gpsimd.sem_clear(sem_range)
        nc._nrt_pseudo_barrier()

    with tile.TileContext(nc) as tc:
        with (
            tc.tile_pool(name="wpool", bufs=1) as wpool,
            tc.tile_pool(name="xpool", bufs=1) as xpool,
            tc.tile_pool(name="hpool", bufs=1) as hpool,
            tc.tile_pool(name="ypool", bufs=1) as ypool,
            tc.tile_pool(name="bpool", bufs=1) as bpool,
            tc.tile_pool(name="psum", bufs=4, space="PSUM") as pspool,
        ):
            w1 = wpool.tile([P, KD, H], mm_dt, tag="w1")
            xt = xpool.tile([P, KD, C], mm_dt, tag="xt")
            w2 = wpool.tile([P, KH, D], mm_dt, tag="w2")
            bias = bpool.tile([P, 2, KH], f32, tag="bias")

            w1_r = w_in_d.rearrange("(k p) h -> p k h", p=P)
            w2_r = w_out_d.rearrange("(k p) h -> p k h", p=P)
            xt_r = xT_d.rearrange("p (k c) -> p k c", c=C)
            yT_r = yT_d.rearrange("p (m c) -> p m c", c=C)

            # Weight/activation streaming over both HWDGE rings.  Each ring
            # sustains ~180 GB/s when both are active (HBM ~358 GB/s/core),
            # and the PE eats a 256KB w1 chunk every ~1.05us -- so w1 chunks
            # alternate k across the rings in JIT consumption order, with
            # the xt chunks interleaved just before the k-pass that reads
            # them.  w2 follows; it is fully hidden under L1 compute.
            # qSP ring (issue cost ~0.65us each; carries the critical first
            # chunks -- xt[0:1] and w1[0] split in half so the k=0 pass
            # unblocks on 128KB instead of 400KB of straggling packets):
            nc.sync.dma_start(xt[:, 0:1, :], xt_r[:, 0:1, :])
            nc.sync.dma_start(w1[:, 0:1, 0 : H // 2], w1_r[:, 0:1, 0 : H // 2])
            nc.sync.dma_start(w1[:, 0:1, H // 2 : H], w1_r[:, 0:1, H // 2 : H])
            nc.sync.dma_start(w1[:, 2:3, :], w1_r[:, 2:3, :])
            nc.sync.dma_start(xt[:, 2:4, :], xt_r[:, 2:4, :])
            nc.sync.dma_start(xt[:, 4:KD, :], xt_r[:, 4:KD, :])
            nc.sync.dma_start(w1[:, 4:5, :], w1_r[:, 4:5, :])
            nc.sync.dma_start(w2[:, 0:2, :], w2_r[:, 0:2, :])
            nc.sync.dma_start(w2[:, 2:4, :], w2_r[:, 2:4, :])
            # qAct ring (issue cost ~0.9us each; fewer, larger items):
            nc.scalar.dma_start(bias[:], bias_d.rearrange("p (b m) -> p b m", m=KH))
            nc.scalar.dma_start(xt[:, 1:2, :], xt_r[:, 1:2, :])
            nc.scalar.dma_start(w1[:, 1:2, :], w1_r[:, 1:2, :])
            nc.scalar.dma_start(w1[:, 3:4, :], w1_r[:, 3:4, :])
            nc.scalar.dma_start(w1[:, 5:6, :], w1_r[:, 5:6, :])
            nc.scalar.dma_start(w1[:, 6:7, :], w1_r[:, 6:7, :])
            nc.scalar.dma_start(w1[:, 7:8, :], w1_r[:, 7:8, :])

            gelu = mybir.ActivationFunctionType.Gelu_apprx_tanh

            # PE warmup during the initial DMA window: dummy matmuls flip
            # the HAM clock gate to 8/8 before the real stream begins.
            wz = bpool.tile([P, P], mm_dt, tag="wz")
            nc.vector.memset(wz[:], 0.0)
            psw = pspool.tile([P, 2, 512], f32, tag="ps", name="ps_warm")
            for i in range(NWARM):
                nc.tensor.matmul(
                    psw[:, 0, :P], wz[:], wz[:], start=(i == 0), stop=(i == NWARM - 1)
                )

            # Dummy activation so the gelu tables load during the DMA
            # window instead of blocking the first real PSUM drain.
            awarm = bpool.tile([P, 1], f32, tag="awarm")
            nc.scalar.activation(awarm[:], wz[:, 0:1], gelu)
            # qAct ring, after the table load: second half of w2.
            nc.scalar.dma_start(w2[:, 4:6, :], w2_r[:, 4:6, :])
            nc.scalar.dma_start(w2[:, 6:8, :], w2_r[:, 6:8, :])

            # layer 1: k-outer for the first KD-2 chunks (matmuls start as
            # soon as the first weight/activation chunks land), then the
            # last two contraction chunks m-outer so the gelu activations
            # overlap layer 1's own tail instead of serializing after it.
            ht = hpool.tile([P, KH, C], mm_dt, tag="ht")
            ps1 = [
                pspool.tile([P, 2, 512], f32, tag="ps", name=f"ps1_{i}")
                for i in range(KH // 2)
            ]
            for k in range(KD - 2):
                for m in range(KH):
                    nc.tensor.matmul(
                        ps1[m // 2][:, m % 2, :C],
                        w1[:, k, m * P : (m + 1) * P],
                        xt[:, k, :],
                        start=(k == 0),
                        stop=False,
                    )
            for m in range(KH):
                for k in (KD - 2, KD - 1):
                    nc.tensor.matmul(
                        ps1[m // 2][:, m % 2, :C],
                        w1[:, k, m * P : (m + 1) * P],
                        xt[:, k, :],
                        start=False,
                        stop=(k == KD - 1),
                    )
                nc.scalar.activation(
                    ht[:, m, :],
                    ps1[m // 2][:, m % 2, :C],
                    gelu,
                    bias=bias[:, 0, m : m + 1],
                )

            # layer 2: k-outer for the first KH-2 chunks, then the last two
            # contraction chunks m-outer so the PSUM pairs complete
            # staggered and the drain overlaps the remaining matmuls.
            yt = ypool.tile([P, KD, C], f16, tag="yt")
            ps2 = [
                pspool.tile([P, 2, 512], f32, tag="ps", name=f"ps2_{i}")
                for i in range(KD // 2)
            ]
            for k in range(KH - 2):
                for m in range(KD):
                    nc.tensor.matmul(
                        ps2[m // 2][:, m % 2, :C],
                        w2[:, k, m * P : (m + 1) * P],
                        ht[:, k, :],
                        start=(k == 0),
                        stop=False,
                    )
            act_copy = mybir.ActivationFunctionType.Copy
            for m in range(KD):
                for k in (KH - 2, KH - 1):
                    nc.tensor.matmul(
                        ps2[m // 2][:, m % 2, :C],
                        w2[:, k, m * P : (m + 1) * P],
                        ht[:, k, :],
                        start=False,
                        stop=(k == KH - 1),
                    )
                if m % 2 == 1:
                    # GpSimd cannot read PSUM on TRN2, so the drain copies
                    # alternate DVE and the Scalar activation path; the last
                    # pair is split bank-wise across both so its drain
                    # latency is halved.
                    j = m // 2
                    if j == KD // 2 - 1:
                        nc.vector.tensor_copy(
                            yt[:, 2 * j : 2 * j + 1, :], ps2[j][:, 0:1, :C]
                        )
                        nc.sync.dma_start(
                            yT_r[:, 2 * j : 2 * j + 1, :], yt[:, 2 * j : 2 * j + 1, :]
                        )
                        nc.scalar.activation(
                            yt[:, 2 * j + 1 : 2 * j + 2, :],
                            ps2[j][:, 1:2, :C],
                            act_copy,
                        )
                        nc.scalar.dma_start(
                            yT_r[:, 2 * j + 1 : 2 * j + 2, :],
                            yt[:, 2 * j + 1 : 2 * j + 2, :],
                        )
                    elif j % 2 == 0:
                        nc.vector.tensor_copy(
                            yt[:, 2 * j : 2 * j + 2, :], ps2[j][:, :, :C]
                        )
                        nc.sync.dma_start(
                            yT_r[:, 2 * j : 2 * j + 2, :], yt[:, 2 * j : 2 * j + 2, :]
                        )
                    else:
                        nc.scalar.activation(
                            yt[:, 2 * j : 2 * j + 2, :], ps2[j][:, :, :C], act_copy
                        )
                        nc.scalar.dma_start(
                            yT_r[:, 2 * j : 2 * j + 2, :], yt[:, 2 * j : 2 * j + 2, :]
                        )

    # The TileContext epilogue emits a Pool-engine Drain with
    # is_reset_sema=True ("dma_reset") + EVENT_SEMAPHORE_RANGE_CLEAR: a
    # serial walk of the DGE ring state that takes a fixed ~7.7us on
    # hardware -- 20% of the whole kernel -- after the last output DMA.
    # The start-side hygiene block above replaces it (each execution cleans
    # before use), so drop both from the END block only.
    if start_clean and os.environ.get("MOE_KEEP_DMA_RESET", "0") != "1":
        blk = nc.m.functions[0].blocks[-1]
        il = blk.instructions
        drop = [
            i
            for i in il
            if ("is_reset_sema=True" in str(i))
            or ("EVENT_SEMAPHORE_RANGE_CLEAR" in str(i))
        ]
        for i in drop:
            il.remove(i)

    nc.compile()
    if not nc.is_finalized():
        nc.finalize()
    return nc


def _get_program(C):
    key = (C, MM_DTYPE)
    if key not in _compiled_cache:
        _compiled_cache[key] = _build_program(C, MM_DTYPE)
    return _compiled_cache[key]


def _route(x2, router_w):
    """Host router in float64: top-2 experts + gate probs per token."""
    logits = x2.astype(np.float64) @ np.asarray(router_w, np.float64)
    logits -= logits.max(axis=-1, keepdims=True)
    ex = np.exp(logits)
    probs = ex / ex.sum(axis=-1, keepdims=True)
    top_e = np.argsort(-probs, axis=-1, kind="stable")[:, :TOPK]  # [T, K]
    top_p = np.take_along_axis(probs, top_e, axis=-1)  # [T, K]
    return top_e, top_p.astype(np.float32)


def kernel(input_batch, router_w, w_in, b_in, w_out, b_out, run_kwargs=None):
    from concourse.bass_utils import run_bass_kernel_spmd

    x = np.ascontiguousarray(np.asarray(input_batch, np.float32))
    B, S, Dm = x.shape
    T = B * S
    x2 = x.reshape(T, Dm)

    top_e, top_p = _route(x2, router_w)

    # per-expert dispatch lists, in expert-sorted (token, k) order like the
    # reference's stable argsort over flattened (token, k) pairs
    tok_lists = [[] for _ in range(E)]
    p_lists = [[] for _ in range(E)]
    for t in range(T):
        for j in range(TOPK):
            e = top_e[t, j]
            tok_lists[e].append(t)
            p_lists[e].append(top_p[t, j])

    counts = [len(l) for l in tok_lists]
    # capacity per wave; a PSUM bank caps the matmul free dim at 512, so an
    # expert with >512 routed tokens (never happens for the spec'd input
    # distribution) is processed in multiple SPMD waves
    n_waves = max(1, -(-max(counts) // 512))
    if n_waves == 1:
        C = max(256, -(-max(counts) // 8) * 8)  # multiple of 8
    else:
        C = 512

    nc = _get_program(C)
    mm_np = _np_mm_dtype(MM_DTYPE)
    KD = D // P

    w_in = np.asarray(w_in, np.float32)
    w_out = np.asarray(w_out, np.float32)
    b_in = np.asarray(b_in, np.float32)
    b_out = np.asarray(b_out, np.float32)

    out = np.zeros((T, Dm), np.float32)
    for w in range(n_waves):
        in_maps = []
        for e in range(E):
            idx = np.asarray(tok_lists[e][w * C : (w + 1) * C], np.int64)
            xT = np.zeros((D, C), mm_np)
            if len(idx):
                xT[:, : len(idx)] = x2[idx].T.astype(mm_np)
            # pack [D, C] -> [128, KD*C] so DMA lines are contiguous
            xTp = np.ascontiguousarray(
                xT.reshape(KD, P, C).transpose(1, 0, 2).reshape(P, KD * C)
            )
            biasP = np.ascontiguousarray(
                np.stack([b_in[e], b_out[e]])
                .reshape(2, KD, P)
                .transpose(2, 0, 1)
                .reshape(P, 2 * KD)
            )
            in_maps.append(
                {
                    "xT": xTp,
                    "w_in": np.ascontiguousarray(w_in[e]).astype(mm_np),
                    "w_out": np.ascontiguousarray(w_out[e]).astype(mm_np),
                    "bias": biasP,
                }
            )

        res = run_bass_kernel_spmd(
            nc, in_maps, core_ids=list(range(N_CORES)), **(run_kwargs or {})
        )
        kernel.last_results = res

        for e in range(E):
            idx = np.asarray(tok_lists[e][w * C : (w + 1) * C], np.int64)
            n = len(idx)
            if n == 0:
                continue
            p = np.asarray(p_lists[e][w * C : (w + 1) * C], np.float32)
            # unpack [128, KD*C] -> [D, C]
            yT = (
                res.results[e]["yT"]
                .astype(np.float32)
                .reshape(P, KD, C)
                .transpose(1, 0, 2)
                .reshape(D, C)
            )
            y = (yT[:, :n].T + b_out[e]) * p[:, None]
            np.add.at(out, idx, y)

    return out.reshape(B, S, Dm)
